# revision 7
# baseline (speedup 1.0000x reference)
"""MoE block (8 experts, top-2) on 8 Trainium2 NeuronCores.

Strategy: expert parallelism. The gate (x @ Wg + bg, 0.01% of total FLOPs)
plus top-2 routing runs on the host as part of the sharding step; each of
the 8 cores then runs one expert's FFN over that expert's tokens:

    yT_e = (relu(X_e @ W1[e] + b1[e]) @ W2[e] + b2[e])^T

Device-side layout keeps activations transposed ([feature, token]) so both
matmuls use natural weight layouts as the stationary operand:

    H^T = W1^T X^T   (contract d=1024,  8 k-tiles)
    Y^T = W2^T H^T   (contract dff=4096, 32 k-tiles)

Matmuls run as float32r (full PE rate, ~2e-4 rel err vs f32). X^T and H^T
stay resident in SBUF for the whole token capacity while W1 and W2 each
stream from HBM exactly once (phase-split). W2 stream buffers reuse the
X^T SBUF slots, which are dead after phase 1. The host applies the top-2
softmax weights and scatters back.
"""

import numpy as np

import concourse.bacc as bacc
import concourse.mybir as mybir
from concourse.tile import TileContext
from concourse.bass_utils import run_bass_kernel_spmd

D = 1024
DFF = 4096
E = 8
TOPK = 2
KD = D // 128      # 8   k-tiles for phase 1
MF = DFF // 128    # 32  dff tiles (phase-1 output / phase-2 contraction)
KF = DFF // 128    # 32
MD = D // 128      # 8   output d tiles

F32 = mybir.dt.float32
F32R = mybir.dt.float32r

# hts (f32) for capacity C needs 32*C*4 bytes/partition; keep C under this
# so the single-weight-pass layout fits SBUF (~208 KiB/partition usable).
MAX_SINGLEPASS_C = 1200

_KERNEL_CACHE = {}


def _build_singlepass(C, NB, nblk):
    """Per-core program, whole capacity resident: phase 1 (stream W1 once)
    then phase 2 (stream W2 once, reusing the X^T SBUF slots)."""
    assert nblk * NB == C

    nc = bacc.Bacc(None, target_bir_lowering=False)
    xT = nc.dram_tensor("xT", [D, C], F32R, kind="ExternalInput")
    w1 = nc.dram_tensor("w1", [MF, 128, KD, 128], F32R, kind="ExternalInput")
    b1c = nc.dram_tensor("b1c", [128, MF], F32, kind="ExternalInput")
    w2 = nc.dram_tensor("w2", [MD, 128, KF, 128], F32R, kind="ExternalInput")
    b2c = nc.dram_tensor("b2c", [128, MD], F32, kind="ExternalInput")
    yT = nc.dram_tensor("yT", [D, C], F32, kind="ExternalOutput")

    WP = 4             # k-tiles per weight stream piece ([128, WP, 128])

    with TileContext(nc) as tc:
        with (
            tc.tile_pool(name="acts", bufs=1) as acts,
            tc.tile_pool(name="wpool", bufs=1) as wpool,
            tc.tile_pool(name="cpool", bufs=1) as cpool,
            tc.tile_pool(name="opool", bufs=1) as opool,
            tc.tile_pool(name="psum", bufs=8, space="PSUM") as psum,
        ):
            # biases via gpsimd (SWDGE) — keeps both HWDGE queues free for
            # the latency-critical streams.
            b1t = cpool.tile([128, MF], F32, name="b1t")
            nc.gpsimd.dma_start(out=b1t[:], in_=b1c[:])
            b2t = cpool.tile([128, MD], F32, name="b2t")
            nc.gpsimd.dma_start(out=b2t[:], in_=b2c[:])

            # X^T resident tiles on the scalar HWDGE queue (weights use the
            # sync queue). Columns land block-by-block across all k so the
            # first matmul groups can start ~1.5 MB into the transfer.
            xts = [
                acts.tile([128, C], F32R, name=f"xt{k}", tag=f"xt{k}")
                for k in range(KD)
            ]
            for nb in range(nblk):
                ns = slice(nb * NB, (nb + 1) * NB)
                for k in range(KD):
                    nc.scalar.dma_start(
                        out=xts[k][:, ns], in_=xT[k * 128:(k + 1) * 128, ns]
                    )

            # PE warm-up: ~5 us of dependency-free dummy matmuls run while
            # the first DMAs land, so HAM unthrottles before real work.
            warm = cpool.tile([128, 128], F32, name="warm")
            nc.any.memset(warm[:], 0.0)
            wps = psum.tile([128, 512], F32, name="wps", tag="ps")
            for _ in range(48):
                nc.tensor.matmul(
                    wps[:, :128], lhsT=warm[:], rhs=warm[:],
                    start=True, stop=True,
                )

            hts = [
                acts.tile([128, C], F32R, name=f"ht{m}", tag=f"ht{m}")
                for m in range(MF)
            ]

            # One [128, WP, 128] weight-piece ring feeds both phases, so the
            # first W2 pieces prefetch while phase 1 finishes.
            def wpiece(src, idx):
                wp = wpool.tile([128, WP, 128], F32R, name="wp", tag="wp",
                                bufs=10)
                nc.sync.dma_start(
                    out=wp[:], in_=src[:, idx * WP:(idx + 1) * WP, :]
                )
                return wp

            # phase 1: H^T[m] = relu(sum_k W1[k,m]^T @ X^T[k] + b1[m])
            for m in range(MF):
                pieces = [wpiece(w1[m], p) for p in range(KD // WP)]
                for nb in range(nblk):
                    ns = slice(nb * NB, (nb + 1) * NB)
                    ps = psum.tile([128, 512], F32, name="ps", tag="ps")[:, :NB]
                    for k in range(KD):
                        nc.tensor.matmul(
                            ps, lhsT=pieces[k // WP][:, k % WP, :],
                            rhs=xts[k][:, ns],
                            start=(k == 0), stop=(k == KD - 1),
                        )
                    nc.scalar.activation(
                        hts[m][:, ns], ps,
                        mybir.ActivationFunctionType.Relu,
                        bias=b1t[:, m:m + 1],
                    )

            # phase 2: Y^T[mo] = sum_k W2[k,mo]^T @ H^T[k] + b2[mo]
            for mo in range(MD):
                pieces = [wpiece(w2[mo], p) for p in range(KF // WP)]
                for nb in range(nblk):
                    ns = slice(nb * NB, (nb + 1) * NB)
                    ps = psum.tile([128, 512], F32, name="ps2", tag="ps")[:, :NB]
                    for k in range(KF):
                        nc.tensor.matmul(
                            ps, lhsT=pieces[k // WP][:, k % WP, :],
                            rhs=hts[k][:, ns],
                            start=(k == 0), stop=(k == KF - 1),
                        )
                    ot = opool.tile([128, NB], F32, name="ot", tag="ot", bufs=3)
                    nc.scalar.activation(
                        ot[:], ps,
                        mybir.ActivationFunctionType.Identity,
                        bias=b2t[:, mo:mo + 1],
                    )
                    nc.scalar.dma_start(
                        out=yT[mo * 128:(mo + 1) * 128, ns], in_=ot[:]
                    )
    nc.compile()
    return nc


def _build_chunked(C, NB, nblk_chunk):
    """Fallback for capacities too large for the single-pass layout:
    process tokens in chunks, re-streaming W1/W2 per chunk."""
    CH = nblk_chunk * NB
    nchunks = C // CH
    assert nchunks * CH == C

    nc = bacc.Bacc(None, target_bir_lowering=False)
    xT = nc.dram_tensor("xT", [D, C], F32R, kind="ExternalInput")
    w1 = nc.dram_tensor("w1", [MF, 128, KD, 128], F32R, kind="ExternalInput")
    b1c = nc.dram_tensor("b1c", [128, MF], F32, kind="ExternalInput")
    w2 = nc.dram_tensor("w2", [MD, 128, KF, 128], F32R, kind="ExternalInput")
    b2c = nc.dram_tensor("b2c", [128, MD], F32, kind="ExternalInput")
    yT = nc.dram_tensor("yT", [D, C], F32, kind="ExternalOutput")

    with TileContext(nc) as tc:
        with (
            tc.tile_pool(name="acts", bufs=1) as acts,
            tc.tile_pool(name="wpool", bufs=1) as wpool,
            tc.tile_pool(name="cpool", bufs=1) as cpool,
            tc.tile_pool(name="opool", bufs=1) as opool,
            tc.tile_pool(name="psum", bufs=8, space="PSUM") as psum,
        ):
            b1t = cpool.tile([128, MF], F32, name="b1t")
            nc.sync.dma_start(out=b1t[:], in_=b1c[:])
            b2t = cpool.tile([128, MD], F32, name="b2t")
            nc.sync.dma_start(out=b2t[:], in_=b2c[:])

            for ci in range(nchunks):
                c0 = ci * CH
                xts = []
                for k in range(KD):
                    t = acts.tile([128, CH], F32R, name=f"xt{k}", tag=f"xt{k}")
                    nc.scalar.dma_start(
                        out=t[:], in_=xT[k * 128:(k + 1) * 128, c0:c0 + CH]
                    )
                    xts.append(t)
                hts = [
                    acts.tile([128, CH], F32R, name=f"ht{m}", tag=f"ht{m}")
                    for m in range(MF)
                ]
                for m in range(MF):
                    w1t = wpool.tile(
                        [128, KD, 128], F32R, name="w1t", tag="w1t", bufs=3
                    )
                    nc.sync.dma_start(out=w1t[:], in_=w1[m])
                    for nb in range(nblk_chunk):
                        ns = slice(nb * NB, (nb + 1) * NB)
                        ps = psum.tile([128, 512], F32, name="ps", tag="ps")[:, :NB]
                        for k in range(KD):
                            nc.tensor.matmul(
                                ps, lhsT=w1t[:, k, :], rhs=xts[k][:, ns],
                                start=(k == 0), stop=(k == KD - 1),
                            )
                        nc.scalar.activation(
                            hts[m][:, ns], ps,
                            mybir.ActivationFunctionType.Relu,
                            bias=b1t[:, m:m + 1],
                        )
                for mo in range(MD):
                    w2t = wpool.tile(
                        [128, KF, 128], F32R, name="w2t", tag="w2t", bufs=2
                    )
                    nc.sync.dma_start(out=w2t[:], in_=w2[mo])
                    for nb in range(nblk_chunk):
                        ns = slice(nb * NB, (nb + 1) * NB)
                        ps = psum.tile([128, 512], F32, name="ps2", tag="ps")[:, :NB]
                        for k in range(KF):
                            nc.tensor.matmul(
                                ps, lhsT=w2t[:, k, :], rhs=hts[k][:, ns],
                                start=(k == 0), stop=(k == KF - 1),
                            )
                        ot = opool.tile(
                            [128, NB], F32, name="ot", tag="ot", bufs=4
                        )
                        nc.scalar.activation(
                            ot[:], ps,
                            mybir.ActivationFunctionType.Identity,
                            bias=b2t[:, mo:mo + 1],
                        )
                        nc.scalar.dma_start(
                            out=yT[mo * 128:(mo + 1) * 128,
                                   c0 + nb * NB:c0 + (nb + 1) * NB],
                            in_=ot[:],
                        )
    nc.compile()
    return nc


def _plan(maxc):
    """Pick capacity/tiling. Blocks must be <= 512 (one PSUM bank of f32)
    and >= 256 (full fp32r rate)."""
    nblk = max(1, -(-maxc // 512))
    NB = max(256, -(-maxc // nblk))
    C = nblk * NB
    if C <= MAX_SINGLEPASS_C:
        return ("single", C, NB, nblk)
    # chunked fallback: 2 blocks per chunk
    NB = max(256, min(512, -(-maxc // 4 // 32) * 32))
    CH = 2 * NB
    nchunks = -(-maxc // CH)
    return ("chunked", nchunks * CH, NB, 2)


def _get_kernel(plan):
    if plan not in _KERNEL_CACHE:
        kind, C, NB, nblk = plan
        if kind == "single":
            _KERNEL_CACHE[plan] = _build_singlepass(C, NB, nblk)
        else:
            _KERNEL_CACHE[plan] = _build_chunked(C, NB, nblk)
    return _KERNEL_CACHE[plan]


def kernel(x, Wg, bg, W1, b1, W2, b2):
    x = np.asarray(x, dtype=np.float32)
    Wg = np.asarray(Wg, dtype=np.float32)
    bg = np.asarray(bg, dtype=np.float32)
    W1 = np.asarray(W1, dtype=np.float32)
    b1 = np.asarray(b1, dtype=np.float32)
    W2 = np.asarray(W2, dtype=np.float32)
    b2 = np.asarray(b2, dtype=np.float32)

    fsz = x.shape[:-1]
    xf = x.reshape(-1, D)
    n = xf.shape[0]

    # ---- routing (host): gate -> top-2 -> softmax over the top-2 ----
    gate = xf @ Wg + bg                                   # [N, E] f32
    top2 = np.argsort(-gate, axis=-1, kind="stable")[:, :TOPK]   # desc, ties->low idx
    vals = np.take_along_axis(gate, top2, axis=-1)        # [N, 2] sorted desc
    ex = np.exp(vals - vals[:, :1])
    wts = ex / ex.sum(axis=-1, keepdims=True)             # [N, 2] f32

    idx_lists = []
    wt_lists = []
    counts = np.zeros(E, dtype=np.int64)
    for e in range(E):
        tok, slot = np.nonzero(top2 == e)
        idx_lists.append(tok)
        wt_lists.append(wts[tok, slot])
        counts[e] = tok.shape[0]
    maxc = int(counts.max())

    plan = _plan(maxc)
    C = plan[1]
    nc = _get_kernel(plan)

    # ---- shard: gather tokens + pre-tile weights per expert ----
    in_maps = []
    for e in range(E):
        xe = np.zeros((C, D), dtype=np.float32)
        xe[:counts[e]] = xf[idx_lists[e]]
        xT = np.ascontiguousarray(xe.T)                     # [D, C]
        w1h = np.ascontiguousarray(
            W1[e].reshape(KD, 128, MF, 128).transpose(2, 1, 0, 3)
        )                                                   # [MF,128,KD,128]
        w2h = np.ascontiguousarray(
            W2[e].reshape(KF, 128, MD, 128).transpose(2, 1, 0, 3)
        )                                                   # [MD,128,KF,128]
        b1c = np.ascontiguousarray(b1[e].reshape(MF, 128).T)  # [128, MF]
        b2c = np.ascontiguousarray(b2[e].reshape(MD, 128).T)  # [128, MD]
        in_maps.append(
            {"xT": xT, "w1": w1h, "b1c": b1c, "w2": w2h, "b2c": b2c}
        )

    res = run_bass_kernel_spmd(nc, in_maps, core_ids=list(range(E)))

    # ---- combine (host): apply top-2 softmax weights, scatter-add ----
    out = np.zeros((n, D), dtype=np.float32)
    for e in range(E):
        ye = res.results[e]["yT"].T[:counts[e]]             # [count, D]
        out[idx_lists[e]] += wt_lists[e][:, None] * ye
    return out.reshape(*fsz, D)
